# revision 22
# baseline (speedup 1.0000x reference)
"""Trainium2 Bass kernel for nn_DEFNet: 16-branch 1D conv (k=3..33) + bias + ReLU
+ channel-mean over x[32, 1, 262144] -> out[32, 262144].

Strategy (per core, 8 cores, 4 batch rows each):
  - Host builds a transposed sliding-window view xwinT[k, t] = xpad[64t + k]
    (k in [0,96)), a constant-ones row 96 (bias), and zero rows 97-111
    (pad to a multiple of 16 partitions so DMA descriptors spread across
    all 16 SDMA engines). Each channel-pair's conv AND bias is ONE matmul:
       psum[(c,p), t] = sum_k lhsT[k, 64c+p] * xwinT[k, t]
    with lhsT[k, 64c+p] = w_masked[2j+c, k-p]/16 (k<96) and
    lhsT[96, 64c+p] = b[2j+c]/16 (mean folded into weights).
  - Per 1024-segment block, 8 pair-tiles stream through 4 PSUM buffers
    (2 N=512 matmuls each). ScalarE relus pairs 0-3 to bf16 (r0, r1 land
    directly in the out tile); VectorE runs two fused max(ps,0)+acc chains
    over pairs 4,6 and 5,7 reading PSUM directly, seeded with scalar tiles
    r2 / r3 — no merge ops and no GpSimd at all. The four bf16 partials
    (r0, r1, chainA, chainB) sit side by side in one out tile, DMA'd with
    a single transfer per block; the host sums partials, folds the two
    64-row halves, and transposes to natural order.
"""

import os

import numpy as np

import concourse.bass as bass
import concourse.mybir as mybir
import concourse.tile as tile
from concourse import bacc, bass_utils
from concourse.tile import TileContext

B, L = 32, 262144
NCONV, MAXK = 16, 33
NCORES = 8
ROWS = B // NCORES          # batch rows per core
P = 64                      # output positions per segment
W = 112                     # window rows (96 data + bias row 96 + zero pad)
HALO = 16
T = L // P                  # segments per row (4096)

# --- tunables -------------------------------------------------------------
BLK = 1024                  # segments per block (2 psum banks)
MMN = 512                   # matmul N cap (one 2KB psum bank of f32)
XBLK = 2048                 # segments per x-in DMA (2 blocks)
DT_X = mybir.dt.float16
DT_W = mybir.dt.float16
DT_E = mybir.dt.bfloat16    # relu/accumulate dtype
F32 = mybir.dt.float32

NPART = 4                   # partial tiles per block across the two out tiles


def _support_mask():
    m = np.zeros((NCONV, MAXK), dtype=np.float32)
    c = MAXK // 2
    for i in range(1, NCONV + 1):
        m[i - 1, c - i:c + i + 1] = 1.0
    return m


def _build_lhsT(w, b):
    """[112, 8*128] f32; pair j cols j*128..(j+1)*128,
    lhsT[k, 64c+p] = wm[2j+c, k-p]/16 (k<96), lhsT[96, ...] = b/16."""
    wm = (np.asarray(w, np.float32) * _support_mask()) / 16.0
    bs = np.asarray(b, np.float32) / 16.0
    lhsT = np.zeros((W, 8 * 128), dtype=np.float32)
    for j in range(8):
        for c in range(2):
            ch = 2 * j + c
            for p in range(P):
                lhsT[p:p + MAXK, j * 128 + c * 64 + p] = wm[ch]
            lhsT[96, j * 128 + c * 64:j * 128 + c * 64 + P] = bs[ch]
    return lhsT


def _build_nc():
    nc = bacc.Bacc(
        "TRN2",
        target_bir_lowering=False,
        debug=False,
        enable_asserts=False,
        num_devices=NCORES,
    )
    xwin = nc.dram_tensor("xwin", [ROWS * W, T], DT_X, kind="ExternalInput").ap()
    wts = nc.dram_tensor("wts", [W, 8 * 128], DT_W, kind="ExternalInput").ap()
    outH = nc.dram_tensor(
        "outH", [ROWS * 128, NPART * T], DT_E, kind="ExternalOutput").ap()

    n_blk = T // BLK
    relu = mybir.ActivationFunctionType.Relu
    op_max, op_add = mybir.AluOpType.max, mybir.AluOpType.add

    with TileContext(nc) as tc:
        with (
            tc.tile_pool(name="consts", bufs=1) as cpool,
            tc.tile_pool(name="xin", bufs=4) as xpool,
            tc.tile_pool(name="psum", bufs=4, space="PSUM") as pspool,
            tc.tile_pool(name="relu", bufs=8) as rpool,
            tc.tile_pool(name="acc", bufs=8) as apool,
            tc.tile_pool(name="out", bufs=6) as opool,
        ):
            w_sb = cpool.tile([W, 8 * 128], DT_W)
            nc.sync.dma_start(w_sb[:], wts[:])
            # warm scalar/vector views of w_sb so later ops carry fewer
            # distinct sync waits per instruction
            warm = cpool.tile([W, 8], DT_W)
            nc.vector.tensor_copy(out=warm[:], in_=w_sb[:, 0:8])
            warm3 = cpool.tile([W, 8], DT_W)
            nc.scalar.copy(warm3[:], w_sb[:, 0:8])

            for r in range(ROWS):
                for xb in range(T // XBLK):
                    x_sb = xpool.tile([W, XBLK], DT_X)
                    nc.sync.dma_start(
                        x_sb[:], xwin[r * W:(r + 1) * W,
                                      xb * XBLK:(xb + 1) * XBLK])
                    for sub in range(XBLK // BLK):
                        blk = xb * (XBLK // BLK) + sub
                        s0 = blk * BLK
                        xs = x_sb[:, sub * BLK:(sub + 1) * BLK]
                        # per-engine out tiles avoid cross-engine semaphore
                        # spacing (event-accel workaround) on a shared tile
                        otS = opool.tile([128, 2 * BLK], DT_E, tag="otS")
                        otV = opool.tile([128, 2 * BLK], DT_E, tag="otV")
                        # scalar front-loaded, vector back-loaded: the
                        # engines phase-shift half a block, hiding the
                        # seed-ACT wait at each block start
                        order = [2, 3, 0, 4, 1, 5, 6, 7]
                        rts = {}
                        cA = cB = None
                        for j in order:
                            lhsT = w_sb[:, j * 128:(j + 1) * 128]
                            ps = pspool.tile([128, BLK], F32)
                            for m in range(BLK // MMN):
                                nc.tensor.matmul(
                                    ps[:, m * MMN:(m + 1) * MMN], lhsT,
                                    xs[:, m * MMN:(m + 1) * MMN],
                                    start=True, stop=True)
                            if j in (2, 3):
                                rt = rpool.tile([128, BLK], DT_E)
                                nc.scalar.activation(rt[:], ps[:], relu)
                                rts[j] = rt
                            elif j in (0, 1):
                                nc.scalar.activation(
                                    otS[:, j * BLK:(j + 1) * BLK], ps[:],
                                    relu)
                            elif j == 4:
                                cA = apool.tile([128, BLK], DT_E, tag="cA")
                                nc.vector.scalar_tensor_tensor(
                                    cA[:], ps[:], 0.0, rts[2][:],
                                    op_max, op_add)
                            elif j == 5:
                                cB = apool.tile([128, BLK], DT_E, tag="cB")
                                nc.vector.scalar_tensor_tensor(
                                    cB[:], ps[:], 0.0, rts[3][:],
                                    op_max, op_add)
                            elif j == 6:
                                nc.vector.scalar_tensor_tensor(
                                    otV[:, 0:BLK], ps[:], 0.0,
                                    cA[:], op_max, op_add)
                            else:
                                nc.vector.scalar_tensor_tensor(
                                    otV[:, BLK:2 * BLK], ps[:], 0.0,
                                    cB[:], op_max, op_add)
                        c0 = NPART * s0
                        nc.sync.dma_start(
                            outH[r * 128:(r + 1) * 128,
                                 c0:c0 + 2 * BLK], otS[:])
                        nc.sync.dma_start(
                            outH[r * 128:(r + 1) * 128,
                                 c0 + 2 * BLK:c0 + 4 * BLK], otV[:])
    nc.compile()
    return nc


_NC_CACHE = None


def _get_nc():
    global _NC_CACHE
    if _NC_CACHE is None:
        _NC_CACHE = _build_nc()
    return _NC_CACHE


LAST_RESULTS = None


def _install_ntff_hook():
    """Provide antenv.axon_hooks (absent on this image) so
    run_bass_kernel_spmd(trace=True) can capture NTFF profiles via the
    axon PJRT plugin's C ABI. Also stub the artifact upload (no bucket
    creds in-container)."""
    import contextlib
    import ctypes
    import sys
    import types

    try:
        from antenv.axon_hooks import get_axon_ntff_profile_hook  # noqa: F401
        return  # real module present
    except ImportError:
        pass

    so_path = "/opt/axon/libaxon_pjrt.so"
    lib = ctypes.CDLL(so_path)
    lib.axon_start_nrt_profile.argtypes = [
        ctypes.POINTER(ctypes.c_int64), ctypes.c_size_t]
    lib.axon_start_nrt_profile.restype = ctypes.c_int64
    lib.axon_stop_nrt_profile.argtypes = [ctypes.c_char_p]
    lib.axon_stop_nrt_profile.restype = ctypes.c_int64

    @contextlib.contextmanager
    def _hook(output_dir, device_ids):
        import jax
        jax.devices()
        if device_ids:
            ids = (ctypes.c_int64 * len(device_ids))(*device_ids)
            rc = lib.axon_start_nrt_profile(ids, len(device_ids))
        else:
            rc = lib.axon_start_nrt_profile(None, 0)
        if rc != 0:
            raise RuntimeError(f"axon_start_nrt_profile rc={rc}")
        try:
            yield
        finally:
            n = lib.axon_stop_nrt_profile(str(output_dir).encode())
            print(f"ntff profile: {n} file(s) -> {output_dir}")

    mod = types.ModuleType("antenv.axon_hooks")
    mod.get_axon_ntff_profile_hook = lambda: _hook
    mod.set_axon_ntff_profile_hook = lambda h: None
    sys.modules["antenv.axon_hooks"] = mod
    bass_utils.upload_artifacts = lambda tmpdir: f"file://{tmpdir}"


def host_inputs(x, w, b):
    """Build the 8 per-core input maps from the full problem inputs."""
    x = np.asarray(x, np.float32)
    xpad = np.pad(x[:, 0, :], ((0, 0), (HALO, HALO)))  # [B, L+32]
    s = xpad.strides
    np_x = mybir.dt.np(DT_X)
    xwinT = np.lib.stride_tricks.as_strided(
        xpad, shape=(B, 96, T), strides=(s[0], s[1], P * s[1]))

    lhsT = _build_lhsT(w, b).astype(mybir.dt.np(DT_W))

    in_maps = []
    for core in range(NCORES):
        rows = xwinT[core * ROWS:(core + 1) * ROWS]          # [4, 96, T]
        xw = np.zeros((ROWS, W, T), dtype=np_x)
        xw[:, :96, :] = rows
        xw[:, 96, :] = 1.0                                   # bias row
        in_maps.append({
            "xwin": xw.reshape(ROWS * W, T),
            "wts": lhsT,
        })
    return in_maps


def kernel(x, w, b):
    global LAST_RESULTS
    in_maps = host_inputs(x, w, b)
    nc = _get_nc()
    trace = bool(os.environ.get("KERNEL_TRACE"))
    if trace:
        _install_ntff_hook()
    res = bass_utils.run_bass_kernel_spmd(
        nc, in_maps, core_ids=list(range(NCORES)), trace=trace,
        **({"trace_cores": [0]} if trace else {}),
    )
    LAST_RESULTS = res

    n_blk = T // BLK
    out = np.empty((B, L), dtype=np.float32)
    for core in range(NCORES):
        # outH rows: [ROWS, 2, P]; cols: [n_blk, NPART, BLK]
        oH = res.results[core]["outH"].reshape(
            ROWS, 2, P, n_blk, NPART, BLK).astype(np.float32)
        folded = oH.sum(axis=(1, 4))                          # [ROWS, P, n_blk, BLK]
        for r in range(ROWS):
            # position = (blk*BLK + t)*64 + p  ->  [n_blk, BLK, P] order
            out[core * ROWS + r] = folded[r].transpose(1, 2, 0).reshape(L)
    return out


# revision 23
# speedup vs baseline: 1.0247x; 1.0247x over previous
"""Trainium2 Bass kernel for nn_DEFNet: 16-branch 1D conv (k=3..33) + bias + ReLU
+ channel-mean over x[32, 1, 262144] -> out[32, 262144].

Strategy (per core, 8 cores, 4 batch rows each):
  - Host builds a transposed sliding-window view xwinT[k, t] = xpad[64t + k]
    (k in [0,96)), a constant-ones row 96 (bias), and zero rows 97-111
    (pad to a multiple of 16 partitions so DMA descriptors spread across
    all 16 SDMA engines). Each channel-pair's conv AND bias is ONE matmul:
       psum[(c,p), t] = sum_k lhsT[k, 64c+p] * xwinT[k, t]
    with lhsT[k, 64c+p] = w_masked[2j+c, k-p]/16 (k<96) and
    lhsT[96, 64c+p] = b[2j+c]/16 (mean folded into weights).
  - Per 1024-segment block, 8 pair-tiles stream through 4 PSUM buffers
    (2 N=512 matmuls each). ScalarE relus pairs 0-3 to bf16 (r0, r1 land
    directly in the out tile); VectorE runs two fused max(ps,0)+acc chains
    over pairs 4,6 and 5,7 reading PSUM directly, seeded with scalar tiles
    r2 / r3 — no merge ops and no GpSimd at all. The four bf16 partials
    (r0, r1, chainA, chainB) sit side by side in one out tile, DMA'd with
    a single transfer per block; the host sums partials, folds the two
    64-row halves, and transposes to natural order.
"""

import os

import numpy as np

import concourse.bass as bass
import concourse.mybir as mybir
import concourse.tile as tile
from concourse import bacc, bass_utils
from concourse.tile import TileContext

B, L = 32, 262144
NCONV, MAXK = 16, 33
NCORES = 8
ROWS = B // NCORES          # batch rows per core
P = 64                      # output positions per segment
W = 112                     # window rows (96 data + bias row 96 + zero pad)
HALO = 16
T = L // P                  # segments per row (4096)

# --- tunables -------------------------------------------------------------
BLK = 1024                  # segments per block (2 psum banks)
MMN = 512                   # matmul N cap (one 2KB psum bank of f32)
XBLK = 2048                 # segments per x-in DMA (2 blocks)
DT_X = mybir.dt.float16
DT_W = mybir.dt.float16
DT_E = mybir.dt.bfloat16    # relu/accumulate dtype
F32 = mybir.dt.float32

NPART = 4                   # partial tiles per block across the two out tiles


def _support_mask():
    m = np.zeros((NCONV, MAXK), dtype=np.float32)
    c = MAXK // 2
    for i in range(1, NCONV + 1):
        m[i - 1, c - i:c + i + 1] = 1.0
    return m


def _build_lhsT(w, b):
    """[112, 8*128] f32; pair j cols j*128..(j+1)*128,
    lhsT[k, 64c+p] = wm[2j+c, k-p]/16 (k<96), lhsT[96, ...] = b/16."""
    wm = (np.asarray(w, np.float32) * _support_mask()) / 16.0
    bs = np.asarray(b, np.float32) / 16.0
    lhsT = np.zeros((W, 8 * 128), dtype=np.float32)
    for j in range(8):
        for c in range(2):
            ch = 2 * j + c
            for p in range(P):
                lhsT[p:p + MAXK, j * 128 + c * 64 + p] = wm[ch]
            lhsT[96, j * 128 + c * 64:j * 128 + c * 64 + P] = bs[ch]
    return lhsT


def _build_nc():
    nc = bacc.Bacc(
        "TRN2",
        target_bir_lowering=False,
        debug=False,
        enable_asserts=False,
        num_devices=NCORES,
    )
    xwin = nc.dram_tensor("xwin", [ROWS * W, T], DT_X, kind="ExternalInput").ap()
    wts = nc.dram_tensor("wts", [W, 8 * 128], DT_W, kind="ExternalInput").ap()
    outH = nc.dram_tensor(
        "outH", [ROWS * 128, NPART * T], DT_E, kind="ExternalOutput").ap()

    n_blk = T // BLK
    relu = mybir.ActivationFunctionType.Relu
    op_max, op_add = mybir.AluOpType.max, mybir.AluOpType.add

    with TileContext(nc) as tc:
        with (
            tc.tile_pool(name="consts", bufs=1) as cpool,
            tc.tile_pool(name="xin", bufs=4) as xpool,
            tc.tile_pool(name="psum", bufs=4, space="PSUM") as pspool,
            tc.tile_pool(name="relu", bufs=8) as rpool,
            tc.tile_pool(name="acc", bufs=8) as apool,
            tc.tile_pool(name="out", bufs=6) as opool,
        ):
            w_sb = cpool.tile([W, 8 * 128], DT_W)
            nc.sync.dma_start(w_sb[:], wts[:])
            # warm scalar/vector views of w_sb so later ops carry fewer
            # distinct sync waits per instruction
            warm = cpool.tile([W, 8], DT_W)
            nc.vector.tensor_copy(out=warm[:], in_=w_sb[:, 0:8])
            warm3 = cpool.tile([W, 8], DT_W)
            nc.scalar.copy(warm3[:], w_sb[:, 0:8])

            for r in range(ROWS):
                for xb in range(T // XBLK):
                    x_sb = xpool.tile([W, XBLK], DT_X)
                    nc.sync.dma_start(
                        x_sb[:], xwin[r * W:(r + 1) * W,
                                      xb * XBLK:(xb + 1) * XBLK])
                    for sub in range(XBLK // BLK):
                        blk = xb * (XBLK // BLK) + sub
                        s0 = blk * BLK
                        xs = x_sb[:, sub * BLK:(sub + 1) * BLK]
                        # per-engine out tiles avoid cross-engine semaphore
                        # spacing (event-accel workaround) on a shared tile
                        otS = opool.tile([128, 2 * BLK], DT_E, tag="otS")
                        otV = opool.tile([128, 2 * BLK], DT_E, tag="otV")
                        # chain seeds r2/r3 first, then alternate
                        # scalar/vector pairs; two 2-deep vector chains
                        order = [2, 4, 3, 5, 0, 6, 1, 7]
                        rts = {}
                        cA = cB = None
                        for j in order:
                            lhsT = w_sb[:, j * 128:(j + 1) * 128]
                            ps = pspool.tile([128, BLK], F32)
                            for m in range(BLK // MMN):
                                nc.tensor.matmul(
                                    ps[:, m * MMN:(m + 1) * MMN], lhsT,
                                    xs[:, m * MMN:(m + 1) * MMN],
                                    start=True, stop=True)
                            if j in (2, 3):
                                rt = rpool.tile([128, BLK], DT_E)
                                nc.scalar.activation(rt[:], ps[:], relu)
                                rts[j] = rt
                            elif j in (0, 1):
                                nc.scalar.activation(
                                    otS[:, j * BLK:(j + 1) * BLK], ps[:],
                                    relu)
                            elif j == 4:
                                cA = apool.tile([128, BLK], DT_E, tag="cA")
                                nc.vector.scalar_tensor_tensor(
                                    cA[:], ps[:], 0.0, rts[2][:],
                                    op_max, op_add)
                            elif j == 5:
                                cB = apool.tile([128, BLK], DT_E, tag="cB")
                                nc.vector.scalar_tensor_tensor(
                                    cB[:], ps[:], 0.0, rts[3][:],
                                    op_max, op_add)
                            elif j == 6:
                                nc.vector.scalar_tensor_tensor(
                                    otV[:, 0:BLK], ps[:], 0.0,
                                    cA[:], op_max, op_add)
                            else:
                                nc.vector.scalar_tensor_tensor(
                                    otV[:, BLK:2 * BLK], ps[:], 0.0,
                                    cB[:], op_max, op_add)
                        c0 = NPART * s0
                        nc.sync.dma_start(
                            outH[r * 128:(r + 1) * 128,
                                 c0:c0 + 2 * BLK], otS[:])
                        nc.sync.dma_start(
                            outH[r * 128:(r + 1) * 128,
                                 c0 + 2 * BLK:c0 + 4 * BLK], otV[:])
    nc.compile()
    return nc


_NC_CACHE = None


def _get_nc():
    global _NC_CACHE
    if _NC_CACHE is None:
        _NC_CACHE = _build_nc()
    return _NC_CACHE


LAST_RESULTS = None


def _install_ntff_hook():
    """Provide antenv.axon_hooks (absent on this image) so
    run_bass_kernel_spmd(trace=True) can capture NTFF profiles via the
    axon PJRT plugin's C ABI. Also stub the artifact upload (no bucket
    creds in-container)."""
    import contextlib
    import ctypes
    import sys
    import types

    try:
        from antenv.axon_hooks import get_axon_ntff_profile_hook  # noqa: F401
        return  # real module present
    except ImportError:
        pass

    so_path = "/opt/axon/libaxon_pjrt.so"
    lib = ctypes.CDLL(so_path)
    lib.axon_start_nrt_profile.argtypes = [
        ctypes.POINTER(ctypes.c_int64), ctypes.c_size_t]
    lib.axon_start_nrt_profile.restype = ctypes.c_int64
    lib.axon_stop_nrt_profile.argtypes = [ctypes.c_char_p]
    lib.axon_stop_nrt_profile.restype = ctypes.c_int64

    @contextlib.contextmanager
    def _hook(output_dir, device_ids):
        import jax
        jax.devices()
        if device_ids:
            ids = (ctypes.c_int64 * len(device_ids))(*device_ids)
            rc = lib.axon_start_nrt_profile(ids, len(device_ids))
        else:
            rc = lib.axon_start_nrt_profile(None, 0)
        if rc != 0:
            raise RuntimeError(f"axon_start_nrt_profile rc={rc}")
        try:
            yield
        finally:
            n = lib.axon_stop_nrt_profile(str(output_dir).encode())
            print(f"ntff profile: {n} file(s) -> {output_dir}")

    mod = types.ModuleType("antenv.axon_hooks")
    mod.get_axon_ntff_profile_hook = lambda: _hook
    mod.set_axon_ntff_profile_hook = lambda h: None
    sys.modules["antenv.axon_hooks"] = mod
    bass_utils.upload_artifacts = lambda tmpdir: f"file://{tmpdir}"


def host_inputs(x, w, b):
    """Build the 8 per-core input maps from the full problem inputs."""
    x = np.asarray(x, np.float32)
    xpad = np.pad(x[:, 0, :], ((0, 0), (HALO, HALO)))  # [B, L+32]
    s = xpad.strides
    np_x = mybir.dt.np(DT_X)
    xwinT = np.lib.stride_tricks.as_strided(
        xpad, shape=(B, 96, T), strides=(s[0], s[1], P * s[1]))

    lhsT = _build_lhsT(w, b).astype(mybir.dt.np(DT_W))

    in_maps = []
    for core in range(NCORES):
        rows = xwinT[core * ROWS:(core + 1) * ROWS]          # [4, 96, T]
        xw = np.zeros((ROWS, W, T), dtype=np_x)
        xw[:, :96, :] = rows
        xw[:, 96, :] = 1.0                                   # bias row
        in_maps.append({
            "xwin": xw.reshape(ROWS * W, T),
            "wts": lhsT,
        })
    return in_maps


def kernel(x, w, b):
    global LAST_RESULTS
    in_maps = host_inputs(x, w, b)
    nc = _get_nc()
    trace = bool(os.environ.get("KERNEL_TRACE"))
    if trace:
        _install_ntff_hook()
    res = bass_utils.run_bass_kernel_spmd(
        nc, in_maps, core_ids=list(range(NCORES)), trace=trace,
        **({"trace_cores": [0]} if trace else {}),
    )
    LAST_RESULTS = res

    n_blk = T // BLK
    out = np.empty((B, L), dtype=np.float32)
    for core in range(NCORES):
        # outH rows: [ROWS, 2, P]; cols: [n_blk, NPART, BLK]
        oH = res.results[core]["outH"].reshape(
            ROWS, 2, P, n_blk, NPART, BLK).astype(np.float32)
        folded = oH.sum(axis=(1, 4))                          # [ROWS, P, n_blk, BLK]
        for r in range(ROWS):
            # position = (blk*BLK + t)*64 + p  ->  [n_blk, BLK, P] order
            out[core * ROWS + r] = folded[r].transpose(1, 2, 0).reshape(L)
    return out
